# revision 50
# baseline (speedup 1.0000x reference)
"""Trainium2 Bass kernel for nn_Decoder (2-layer LSTM decoder + vocab head).

Computation (matches reference.py):
  embeds = emb[sentence]                      [B, T, E]
  x = concat(features, embeds[:, :-1])        [B, T, E]
  h0 = LSTM0(x), h1 = LSTM1(h0)               [B, T, H]
  out = (h1 @ fc_W.T + fc_b).transpose(0,2,1) [B, V, T]

Sharding (8 NeuronCores, SPMD, no collectives):
  - LSTM replicated on all cores (weight-ingestion bound, batch split
    would not reduce wall time); fc vocab dim sharded 8 ways (4000 rows
    per core, padded to 4096).
  - Per-core logits are written in [V_loc, T, B] layout (2 KB contiguous
    per-partition DMA runs); the host concatenates and transposes.

Device layout ("k-space"): every tensor entering a matmul keeps the
contraction dim on partitions.  Gate chunks land g-on-partitions, so the
LSTM state (c, h) stays k-aligned and feeds the next step's stationary
operand without any transpose.

Key optimizations over the first working version (484 us):
  - fc is fused into the LSTM step loop: the vocab head is computed in
    four 8-step t-blocks, with individual fc matmuls interleaved between
    the recurrent LDWEIGHTS+MATMUL pairs (the rec stream is
    weight-load paced, leaving the MM pipe half idle; fc MMs fill it).
  - W_hh is stored fp8e4 (x16, compensated by writing h/16 into the
    histories and scaling W_ih1 / fc_W by 16): LDWEIGHTS with FWL reads
    fp8 weights faster, and SBUF/DMA shrink.
  - The xp (input-projection) PSUM->SBUF bias folds moved from 256
    per-gate-block ScalarE activations to 128 batched VectorE adds
    against a precomputed broadcast bias tensor.
  - fc bias+staging alternates ScalarE/VectorE; output DMAs trigger on
    the idle Sync/GpSimd engines.
  - A short identity-matmul warmup keeps the PE HAM clock-gate busy
    during the initial weight DMAs.

Environment note: this walrus build rejects >1 embedded sync wait per
instruction; _split_waits_json() rewrites the serialized BIR, hoisting
excess waits onto same-engine NoOp carriers (identical semantics).
"""

import numpy as np
import ml_dtypes

# ---------------------------------------------------------------------------
# Workaround: this walrus build caps instructions at ONE embedded sync wait
# ("Too many sync wait commands" in setupSyncWait); Tile routinely attaches
# several.  Post-process the serialized BIR: hoist excess waits of every
# instruction onto same-engine NoOp carriers inserted immediately before it.
# ---------------------------------------------------------------------------
import orjson
import concourse.tile as tile

_MAXW = 1


def _split_waits_json(b: bytes) -> bytes:
    d = orjson.loads(b)
    for f in d["functions"]:
        for blk in f["blocks"]:
            out = []
            for inst in blk["instructions"]:
                si = inst.get("sync_info")
                if si:
                    w = si.get("on_wait") or []
                    if len(w) > _MAXW:
                        for i, wt in enumerate(w[:-_MAXW]):
                            out.append(
                                {
                                    "debug": inst.get("debug", 0),
                                    "engine": inst["engine"],
                                    "ins": [],
                                    "outs": [],
                                    "name": f"{inst['name']}-hw{i}",
                                    "opcode": "NoOp",
                                    "sync_info": {"on_update": [], "on_wait": [wt]},
                                }
                            )
                        si["on_wait"] = w[-_MAXW:]
                out.append(inst)
            blk["instructions"] = out
    return orjson.dumps(d)


def _patch_serialization(nc):
    orig = nc.to_json_bytes
    nc.to_json_bytes = lambda: _split_waits_json(orig())
    return nc


import concourse.bass as bass
import concourse.mybir as mybir
from concourse.bass import ts, ds
from concourse.bass_utils import run_bass_kernel_spmd

F32 = mybir.dt.float32
BF16 = mybir.dt.bfloat16
FP8 = mybir.dt.float8e4
AF = mybir.ActivationFunctionType
MULT = mybir.AluOpType.mult
ADD = mybir.AluOpType.add
DR = mybir.MatmulPerfMode.DoubleRow
BF16_NP = ml_dtypes.bfloat16
FP8_NP = ml_dtypes.float8_e4m3

E, H, V, B, T = 512, 512, 32000, 64, 32
G = 4 * H                    # 2048 gate rows per layer
KC = 4                       # 512 = 4 k-chunks of 128
NCORES = 8
VPAD = 4096                  # per-core vocab slice, padded from 4000
NTOK = B * T                 # 2048
LAG = 4                      # rec1 runs LAG steps behind rec0
WSC = 16.0                   # fp8 weight scale; histories hold h/WSC
# fc t-blocks (start step, len): finer at the end so the post-loop tail
# has fc work to interleave and a small final drain
TBS = [(0, 4), (4, 4), (8, 8), (16, 8), (24, 4), (28, 4)]


class _FcFiller:
    """Interleaves single fc matmuls into the recurrent instruction stream.

    One (tblock, vblock) job = 4 accumulating matmuls (one per k-chunk,
    N = TB*B = 512) + a bias/stage op + the output DMA.  emit(n) advances
    by up to n matmuls; jobs gate on tblock readiness (set_ready is called
    right after the rec1 step that wrote the last history slice of the
    block, so program order implies the RAW dependency)."""

    def __init__(self, nc, fcw_sb, hist1t, fcb_sb, ps_pool, stage_pool, out_d):
        self.nc = nc
        self.fcw_sb = fcw_sb
        self.hist1t = hist1t
        self.fcb_sb = fcb_sb
        self.ps_pool = ps_pool
        self.stage_pool = stage_pool
        self.out_d = out_d
        self.jobs = [(tb, v) for tb in range(len(TBS)) for v in range(VPAD // 128)]
        self.ready_tb = -1
        self.cur = None
        self.count = 0

    def set_ready(self, tb):
        self.ready_tb = tb

    def emit(self, n):
        nc = self.nc
        for _ in range(n):
            if self.cur is None:
                if not self.jobs or self.jobs[0][0] > self.ready_tb:
                    return
                tb, v = self.jobs.pop(0)
                t0, tl = TBS[tb]
                ps = self.ps_pool.tile([128, 8, B], F32, tag="psfc")
                self.cur = [tb, v, 0, ps]
            tb, v, kc, ps = self.cur
            t0, tl = TBS[tb]
            nc.tensor.matmul(
                ps[:, 0:tl, :],
                self.fcw_sb[:, kc, ts(v, 128)],
                self.hist1t[:, kc, ds(t0, tl), :],
                start=(kc == 0),
                stop=(kc == KC - 1),
                skip_group_check=True,
            )
            if kc == KC - 1:
                ot = self.stage_pool.tile([128, 8, B], F32, tag="ot")
                if self.count % 2 == 0:
                    nc.scalar.activation(
                        out=ot[:, 0:tl, :], in_=ps[:, 0:tl, :], func=AF.Identity,
                        bias=self.fcb_sb[:, v : v + 1], scale=1.0,
                    )
                else:
                    nc.vector.tensor_scalar_add(
                        ot[:, 0:tl, :], ps[:, 0:tl, :], self.fcb_sb[:, v : v + 1]
                    )
                eng = (nc.sync, nc.gpsimd)[self.count % 2]
                eng.dma_start(
                    out=self.out_d[ts(v, 128), ds(t0, tl), :], in_=ot[:, 0:tl, :]
                )
                self.count += 1
                self.cur = None
            else:
                self.cur[2] += 1

    def drain(self):
        while self.jobs or self.cur is not None:
            before = (len(self.jobs), self.cur is None)
            self.emit(64)
            if (len(self.jobs), self.cur is None) == before:
                break  # remaining jobs not ready (should not happen)


def _build_nc():
    nc = bass.Bass()

    # xw = [xT | wih0T], repacked as 8 per-partition-contiguous pieces of
    # [KC, 512] so the startup DMAs have cheap descriptors
    xw_d = nc.dram_tensor("xw", [128, 8, KC, 512], BF16, kind="ExternalInput")
    whh0_d = nc.dram_tensor("whh0T", [128, KC, G], FP8, kind="ExternalInput")
    wih1_d = nc.dram_tensor("wih1T", [128, KC, G], BF16, kind="ExternalInput")
    whh1_d = nc.dram_tensor("whh1T", [128, KC, G], FP8, kind="ExternalInput")
    bb0_d = nc.dram_tensor("bb0", [128, 16, 4, B], BF16, kind="ExternalInput")
    bb1_d = nc.dram_tensor("bb1", [128, 16, 4, B], BF16, kind="ExternalInput")
    ident_d = nc.dram_tensor("ident", [128, 128], BF16, kind="ExternalInput")
    fcw_d = nc.dram_tensor("fcwT", [128, KC, VPAD], BF16, kind="ExternalInput")
    fcb_d = nc.dram_tensor("fcb", [128, VPAD // 128], F32, kind="ExternalInput")
    out_d = nc.dram_tensor("out", [VPAD, T, B], F32, kind="ExternalOutput")

    with tile.TileContext(nc) as tc:
        with (
            tc.tile_pool(name="consts", bufs=1) as consts,
            tc.tile_pool(name="state", bufs=1) as state,
            tc.tile_pool(name="ps_gates", bufs=2, space="PSUM") as ps_gates,
            tc.tile_pool(name="ps_xp", bufs=2, space="PSUM") as ps_xp,
            tc.tile_pool(name="ps_fc", bufs=2, space="PSUM") as ps_fc,
            tc.tile_pool(name="fcstage", bufs=7) as fcstage,
        ):
            # ---- small constants ----
            bb0_sb = consts.tile([128, 16, 4, B], BF16, tag="bb0")
            bb1_sb = consts.tile([128, 16, 4, B], BF16, tag="bb1")
            fcb_sb = consts.tile([128, VPAD // 128], F32, tag="fcb")
            ident = consts.tile([128, 128], BF16, tag="ident")

            # ---- histories (t-major; store h/WSC in bf16) ----
            hist0 = consts.tile([128, KC, T, B], BF16, tag="hist0")
            hist1t = consts.tile([128, KC, T, B], BF16, tag="hist1t")
            # xp rings: [128, slot(2)*16 + g, slab(4), B] bias-folded bf16
            xp0r = consts.tile([128, 32, 4, B], BF16, tag="xp0r")
            xp1r = consts.tile([128, 32, 4, B], BF16, tag="xp1r")

            # ---- per-layer state ----
            st = []
            for l in range(2):
                cT = state.tile([128, KC, B], F32, tag=f"cT{l}", name=f"cT{l}")
                gates = state.tile([128, 16, B], F32, tag=f"gates{l}", name=f"gates{l}")
                tmp1 = state.tile([128, KC, B], F32, tag=f"tmp1{l}", name=f"tmp1{l}")
                tmp2 = state.tile([128, KC, B], F32, tag=f"tmp2{l}", name=f"tmp2{l}")
                tanh_c = state.tile([128, KC, B], F32, tag=f"tanhc{l}", name=f"tanhc{l}")
                st.append(dict(cT=cT, gates=gates, tmp1=tmp1, tmp2=tmp2, tanh_c=tanh_c))

            def xp_chunk(w_slice, rhs_slice, bb_sb, ring, c, fold_scale):
                """Project chunk c (steps 4c..4c+3, 256 tokens) into ring
                slot c%2.  2 gate-blocks per PSUM tile; VectorE folds
                (PSUM*fold_scale)+broadcast-bias -> bf16 ring."""
                s0 = (c % 2) * 16
                for g2 in range(8):
                    ps = ps_xp.tile([128, 2, 4, B], F32, tag="xps")
                    for gg in range(2):
                        gb = 2 * g2 + gg
                        for kc in range(KC):
                            nc.tensor.matmul(
                                ps[:, gg],
                                w_slice(kc, gb),
                                rhs_slice(kc, c),
                                start=(kc == 0),
                                stop=(kc == KC - 1),
                            )
                    nc.vector.scalar_tensor_tensor(
                        ring[:, ds(s0 + 2 * g2, 2), :, :],
                        ps,
                        fold_scale,
                        bb_sb[:, ds(2 * g2, 2), :, :],
                        MULT,
                        ADD,
                    )

            def rec_step(l, t, whh_sb, ring, hist_rd, hist_wr, filler):
                # gate row order (host-reordered): [i(0:4) f(4:8) o(8:12) g(12:16)]
                s = st[l]
                s0 = ((t // 4) % 2) * 16
                sl = t % 4
                ps = ps_gates.tile([128, 16, B], F32, tag="ps01")
                for half in (0, 1):
                    if t > 0:
                        for j in range(8):
                            gc = half * 8 + j
                            for kc in range(KC):
                                nc.tensor.matmul(
                                    ps[:, gc, :],
                                    whh_sb[:, kc, ts(gc, 128)],
                                    hist_rd(kc, t - 1),
                                    start=(j == 0 and kc == 0),
                                    stop=False,
                                    skip_group_check=True,
                                )
                            filler.emit(1)
                    # fold xp(+bias) into the PSUM group via identity weights
                    nc.tensor.matmul(
                        ps[:, ds(half * 8, 8), :],
                        ident,
                        ring[:, ds(s0 + half * 8, 8), sl, :],
                        start=(t == 0),
                        stop=True,
                        skip_group_check=True,
                    )
                filler.emit(2 if t > 0 else 8)
                g = s["gates"]
                nc.scalar.activation(g[:, 0:12, :], ps[:, 0:12, :], func=AF.Sigmoid)
                nc.scalar.activation(g[:, 12:16, :], ps[:, 12:16, :], func=AF.Tanh)
                if t == 0:
                    nc.vector.tensor_mul(s["cT"], g[:, 0:4, :], g[:, 12:16, :])
                else:
                    nc.vector.tensor_mul(s["tmp1"], g[:, 0:4, :], g[:, 12:16, :])
                    nc.vector.tensor_mul(s["tmp2"], g[:, 4:8, :], s["cT"])
                    nc.vector.tensor_add(s["cT"], s["tmp1"], s["tmp2"])
                nc.scalar.activation(s["tanh_c"], s["cT"], func=AF.Tanh)
                # h/WSC = (o * 1/WSC) * tanh(c), one fused VectorE op per dest
                for wr in hist_wr(t):
                    nc.vector.scalar_tensor_tensor(
                        wr, g[:, 8:12, :], 1.0 / WSC, s["tanh_c"], MULT, MULT
                    )

            with tc.tile_pool(name="wpool", bufs=1) as wpool:
                whh0_sb = wpool.tile([128, KC, G], FP8, tag="whh0")
                wih1_sb = wpool.tile([128, KC, G], BF16, tag="wih1")
                whh1_sb = wpool.tile([128, KC, G], FP8, tag="whh1")
                fcw_sb = wpool.tile([128, KC, VPAD], BF16, tag="fcw")

                with tc.tile_pool(name="inpool", bufs=1) as inpool:
                    xw_sb = inpool.tile([128, 8, KC, 512], BF16, tag="xw")

                    # ---- input DMAs (order = criticality: wih0 + first
                    # token quarter first, so xp chunk 0 can start) ----
                    nc.scalar.dma_start(out=ident, in_=ident_d[:])
                    for p in (4, 5):
                        nc.scalar.dma_start(out=xw_sb[:, p], in_=xw_d[:, p])
                    for p in (6, 7):
                        nc.gpsimd.dma_start(out=xw_sb[:, p], in_=xw_d[:, p])
                    nc.sync.dma_start(out=xw_sb[:, 0], in_=xw_d[:, 0])
                    nc.gpsimd.dma_start(out=whh0_sb, in_=whh0_d[:])
                    nc.sync.dma_start(out=bb0_sb, in_=bb0_d[:])
                    for p in (1, 2, 3):
                        nc.scalar.dma_start(out=xw_sb[:, p], in_=xw_d[:, p])
                    nc.sync.dma_start(out=bb1_sb, in_=bb1_d[:])
                    nc.scalar.dma_start(out=fcb_sb, in_=fcb_d[:])

                    # ---- PE warmup against the HAM clock gate: harmless
                    # identity matmuls while the weight DMAs land ----
                    ps_w = ps_fc.tile([128, 8, B], F32, tag="psfc")
                    for _ in range(192):
                        nc.tensor.matmul(
                            ps_w[:, 0, :], ident, ident[:, 0:B],
                            start=True, stop=True, skip_group_check=True,
                        )

                    wih0_w = lambda kc, gb: xw_sb[:, 4 + gb // 4, kc, ds((gb % 4) * 128, 128)]
                    wih1_w = lambda kc, gb: wih1_sb[:, kc, ts(gb, 128)]
                    xp0_rhs = lambda kc, c: xw_sb[:, c // 2, kc, ds((c % 2) * 256, 256)]
                    xp1_rhs = lambda kc, c: hist0[:, kc, ds(4 * c, 4), :]

                    rec0 = dict(
                        whh_sb=whh0_sb,
                        ring=xp0r,
                        hist_rd=lambda kc, t: hist0[:, kc, t, :],
                        hist_wr=lambda t: [hist0[:, :, t, :]],
                    )
                    rec1 = dict(
                        whh_sb=whh1_sb,
                        ring=xp1r,
                        hist_rd=lambda kc, t: hist1t[:, kc, t, :],
                        hist_wr=lambda t: [hist1t[:, :, t, :]],
                    )

                    def mark_ready(s_):
                        for i, (t0, tl) in enumerate(TBS):
                            if s_ == t0 + tl - 1:
                                filler.set_ready(i)

                    filler = _FcFiller(
                        nc, fcw_sb, hist1t, fcb_sb, ps_fc, fcstage, out_d
                    )

                    xp_chunk(wih0_w, xp0_rhs, bb0_sb, xp0r, 0, 1.0)
                    for t in range(T):
                        if t == 0:
                            nc.gpsimd.dma_start(out=wih1_sb, in_=wih1_d[:])
                        if t == 1:
                            nc.gpsimd.dma_start(out=whh1_sb, in_=whh1_d[:])
                        if t == 2:
                            for piece in range(4):
                                nc.gpsimd.dma_start(
                                    out=fcw_sb[:, :, ts(piece, 1024)],
                                    in_=fcw_d[:, :, ts(piece, 1024)],
                                )
                        rec_step(0, t, **rec0, filler=filler)
                        if t % 4 == 1 and t // 4 + 1 <= 7:
                            xp_chunk(wih0_w, xp0_rhs, bb0_sb, xp0r, t // 4 + 1, 1.0)
                        if t % 4 == 3:
                            xp_chunk(wih1_w, xp1_rhs, bb1_sb, xp1r, t // 4, 1.0)
                        if t >= LAG:
                            s_ = t - LAG
                            rec_step(1, s_, **rec1, filler=filler)
                            mark_ready(s_)
                for s_ in range(T - LAG, T):
                    rec_step(1, s_, **rec1, filler=filler)
                    mark_ready(s_)
                    filler.emit(8)
                filler.drain()
    return _patch_serialization(nc)


def _to_k128(W, dtype):
    """W [out_dim, K] -> [128, K//128, out_dim] with result[p,kc,g]=W[g,kc*128+p]."""
    K = W.shape[1]
    return np.ascontiguousarray(
        W.T.reshape(K // 128, 128, -1).transpose(1, 0, 2)
    ).astype(dtype)


_NC_CACHE = None
RUN_KWARGS = {}
LAST_RESULT = None


def kernel(
    sentence,
    features,
    lengths,
    emb,
    W_ih0,
    W_hh0,
    b_ih0,
    b_hh0,
    W_ih1,
    W_hh1,
    b_ih1,
    b_hh1,
    fc_W,
    fc_b,
):
    global _NC_CACHE, LAST_RESULT
    sentence = np.asarray(sentence).astype(np.int64)
    features = np.asarray(features, dtype=np.float32)
    emb = np.asarray(emb, dtype=np.float32)

    # embedding gather + teacher forcing shift (host; pure data movement)
    embeds = emb[sentence[:, : T - 1]]                      # [B, T-1, E]
    x = np.concatenate([features[:, None, :], embeds], axis=1)  # [B, T, E]
    # token-major [k, tok] with tok = t*B + b
    xT = np.ascontiguousarray(x.transpose(2, 1, 0).reshape(E, NTOK))
    xT_p = np.ascontiguousarray(
        xT.reshape(KC, 128, NTOK).transpose(1, 0, 2)
    ).astype(BF16_NP)

    def regate(W):
        """PyTorch gate row order [i f g o] -> ours [i f o g]."""
        W = np.asarray(W, np.float32)
        return np.concatenate([W[: 2 * H], W[3 * H :], W[2 * H : 3 * H]], axis=0)

    wih0 = _to_k128(regate(W_ih0), BF16_NP)
    whh0 = _to_k128(regate(W_hh0) * WSC, FP8_NP)
    wih1 = _to_k128(regate(W_ih1) * WSC, BF16_NP)
    whh1 = _to_k128(regate(W_hh1) * WSC, FP8_NP)
    pieces = lambda a: a.reshape(128, KC, 4, 512).transpose(0, 2, 1, 3)
    xw = np.ascontiguousarray(
        np.concatenate([pieces(xT_p), pieces(wih0)], axis=1)
    )

    def bias_bcast(b_ih, b_hh):
        b = regate(np.asarray(b_ih, np.float32) + np.asarray(b_hh, np.float32))
        b = b.reshape(16, 128).T                      # [128, 16]
        return np.ascontiguousarray(
            np.broadcast_to(b[:, :, None, None], (128, 16, 4, B))
        ).astype(BF16_NP)

    bb0 = bias_bcast(b_ih0, b_hh0)
    bb1 = bias_bcast(b_ih1, b_hh1)

    fc_W = np.asarray(fc_W, np.float32)
    fc_b = np.asarray(fc_b, np.float32)
    vloc = V // NCORES  # 4000 real rows per core, padded to VPAD

    common = {
        "xw": xw,
        "whh0T": whh0,
        "wih1T": wih1,
        "whh1T": whh1,
        "bb0": bb0,
        "bb1": bb1,
        "ident": np.eye(128, dtype=BF16_NP),
    }
    in_maps = []
    for c in range(NCORES):
        wslice = np.zeros((VPAD, H), np.float32)
        wslice[:vloc] = fc_W[c * vloc : (c + 1) * vloc] * WSC
        bslice = np.zeros(VPAD, np.float32)
        bslice[:vloc] = fc_b[c * vloc : (c + 1) * vloc]
        wc = _to_k128(wslice, BF16_NP)
        bc = np.ascontiguousarray(bslice.reshape(VPAD // 128, 128).T)
        in_maps.append({**common, "fcwT": wc, "fcb": bc})

    if _NC_CACHE is None:
        _NC_CACHE = _build_nc()

    res = run_bass_kernel_spmd(
        _NC_CACHE, in_maps, core_ids=list(range(NCORES)), **RUN_KWARGS
    )
    LAST_RESULT = res
    full = np.concatenate(
        [res.results[c]["out"][:vloc] for c in range(NCORES)], axis=0
    )  # [V, T, B]
    return np.ascontiguousarray(full.transpose(2, 0, 1))
